# revision 13
# baseline (speedup 1.0000x reference)
"""Trainium2 Bass kernel for CustomFullyConnectedLayerGoogleTopK2.

Computes out = x @ W.T where
    W[r, c] = alpha_topk[(r-c) % n] * V[(r-c) % n, c]
and alpha_topk is the Dykstra soft-top-k projection of alpha (50 iters in the
reference; the collapsed scalar recursion converges to <1e-5 in 2).

Sharding: output-feature (r) dimension split across 8 NeuronCores (tensor
parallel).  The host linearizes each core's diagonal band of V into a dense
[128, 32*512] bf16 image and ships alpha both in the usual [128, 32] layout
and as a scaled linear+wrap copy (aext = alpha_ext/l, a layout/precast of the
replicated alpha input).  Host concatenates the per-core column slices.

Key structural idea: relu commutes with the circulant gather.  The mask
circulant big[p, m] = mask[(p+m) mod n] = relu(y0[(p+m) mod n] + c) where c
is the scalar Dykstra converges to.  So instead of computing the mask in
[128, 32] layout and bouncing it through DRAM to build the circulant (two
DMA latencies + a slow 128x-redundant read, all on the critical path), the
device loads circulant(y0) straight from aext at body start (overlapped with
everything) and materializes the mask as compute: bmask = relu(big_y0 + c),
per 512-column chunk, the moment Dykstra's scalar lands.

Schedule (from profiling previous versions):
  - Dykstra iteration = fused DVE tensor_scalar (accum_out reduces with op1,
    so relu and row-sum are separate ops) + PE matmul with all-(-1/N)
    weights + DVE copy of c back to SBUF (the DVE/ACT scalar-pointer and
    bias paths cannot read PSUM).
  - bmask chunks alternate scalar (ACT Relu with bias=c) and gpsimd engines;
    the vs multiply (bmask * V-band) runs on vector alone.  Running vector
    and gpsimd concurrently over the SAME tiles makes both 3x slower (SBUF
    port contention), so each engine owns disjoint chunks.
  - The PE pairs PSUM banks 0+1 during the vs trickle: two matmuls per vs
    chunk = PE 100% busy at the vs production rate, which also keeps the HAM
    clock-gate warm (an idle-ish PE is throttled to 1.2GHz).
  - A few warmup matmuls cover the pre-Dykstra window.
  - x streams batch-major; banks 0/1 get quarter-granularity DMAs since the
    pair phase needs both chunks early.  V quads 4-6 ride the ACT ring.
  - All DMAs are pure input loads traced up front -> fresh semaphores, no
    producer-order hazards (Tile deps follow trace order).
  - The whole pipeline runs with the r axis reversed so every DMA access
    pattern has positive steps; the host un-flips the output columns.
"""

import os
import sys

sys.path.insert(0, "/opt/trn_rl_repo")

import numpy as np

N = 4096          # in_features == out_features
B = 1024          # batch rows
P = 128           # partitions
NCORES = 8
RS = N // NCORES  # 512: output columns per core
NCB = N // P      # 32: contraction (c) blocks
NBB = B // P      # 8: batch blocks
KTOP = 41.0
INV_L = 100.0     # 1 / ALPHA_LR
NITER_DEV = 2     # collapsed recursion: mask err 9e-6 by t=2 (bf16 floor 4e-5)
WARM_PRE = 6      # early PE warmups, FD=128

_CACHE = {}


def _build_nc():
    import concourse.bacc as bacc
    import concourse.bass as bass
    import concourse.mybir as mybir
    import concourse.tile as tile
    from concourse.alu_op_type import AluOpType

    f32 = mybir.dt.float32
    bf16 = mybir.dt.bfloat16
    AFT = mybir.ActivationFunctionType
    W32 = N // P  # 32 elements per partition for length-N vectors

    nc = bacc.Bacc("TRN2", debug=False)

    # x host-interleaved batch-major: xb[p, (b*NCB + cb)*P + j] = x[128b+j, 128cb+p]
    xb_d = nc.declare_dram_parameter("xb", [P, NBB * NCB * P], bf16, isOutput=False)
    # V band host-linearized: vb[p, cb*RS + j] = VTk[c, c+j], c = 128cb + p
    vb_d = nc.declare_dram_parameter("vb", [P, NCB * RS], bf16, isOutput=False)
    # aext[i] = alpha[i mod N] / l (linear + wrap), bf16
    ae_d = nc.declare_dram_parameter("aext", [N + 2 * RS], bf16, isOutput=False)
    out_d = nc.declare_dram_parameter("out", [B, RS], f32, isOutput=True)

    XCHUNK = NCB * P  # 4096 columns per batch-block chunk
    XQ = XCHUNK // 4  # quarter chunks for the pair-phase banks
    with tile.TileContext(nc) as tc:
        with (
            tc.tile_pool(name="const", bufs=1) as cpool,
            tc.tile_pool(name="work", bufs=2) as wpool,
            tc.tile_pool(name="bmp", bufs=4) as bmp,
            tc.tile_pool(name="xtp", bufs=1) as xtp,
            tc.tile_pool(name="vtp", bufs=1) as vtp,
            tc.tile_pool(name="vsp", bufs=1) as vsp,
            tc.tile_pool(name="bigp", bufs=1) as bigp,
            tc.tile_pool(name="otp", bufs=2) as otp,
            tc.tile_pool(name="dpsum", bufs=1, space="PSUM") as dpsum,
            tc.tile_pool(name="wupsum", bufs=1, space="PSUM") as wupsum,
            tc.tile_pool(name="mpsum", bufs=2, space="PSUM") as mpsum,
            tc.tile_pool(name="fpsum", bufs=2, space="PSUM") as fpsum,
        ):
            # ---------- input DMAs (all pure loads, traced up front) -------
            # qSP: alpha head first (gates Dykstra), then V quad 0, then the
            # pair-phase x quarters / remaining V quads interleaved by need.
            al_sb = cpool.tile([P, W32], bf16)
            nc.sync.dma_start(
                al_sb[:], ae_d[0:N].rearrange("(p w) -> p w", p=P)
            )
            xts = []
            for b in range(NBB):
                xts.append(xtp.tile([P, XCHUNK], bf16, tag=f"xt{b}", name=f"xt{b}"))
            vts = []
            for g in range(8):
                vts.append(
                    vtp.tile([P, 4 * RS], bf16, tag=f"vt{g}", name=f"vt{g}")
                )

            def vq_load(eng, g):
                eng.dma_start(vts[g][:], vb_d[:, 4 * RS * g : 4 * RS * (g + 1)])

            def xq_load(b, q):
                nc.sync.dma_start(
                    xts[b][:, XQ * q : XQ * (q + 1)],
                    xb_d[:, XCHUNK * b + XQ * q : XCHUNK * b + XQ * (q + 1)],
                )

            vq_load(nc.sync, 0)
            xq_load(0, 0)
            xq_load(1, 0)
            vq_load(nc.sync, 1)
            xq_load(0, 1)
            xq_load(1, 1)
            vq_load(nc.sync, 2)
            xq_load(0, 2)
            xq_load(1, 2)
            vq_load(nc.sync, 3)
            xq_load(0, 3)
            xq_load(1, 3)
            vq_load(nc.sync, 7)
            for b in range(2, NBB):
                nc.sync.dma_start(
                    xts[b][:], xb_d[:, XCHUNK * b : XCHUNK * (b + 1)]
                )
            # qACT: circulant(y0) -- read straight from aext with the
            # overlapping-window pattern; first chunk small so it lands
            # before Dykstra's scalar does.  Then V quads 4-6.
            big = bigp.tile([P, N + RS], bf16)
            big_cuts = [0, 1024, 2560, N + RS]
            for ci in range(3):
                lo, hi = big_cuts[ci], big_cuts[ci + 1]
                nc.scalar.dma_start(
                    big[:, lo:hi],
                    bass.AP(ae_d[:].tensor, lo, [[1, P], [1, hi - lo]]),
                )
            for g in (4, 5, 6):
                vq_load(nc.scalar, g)

            # ---------- constants + early PE warmup ----------
            m3 = cpool.tile([P, P], f32)
            nc.vector.memset(m3[:], -1.0 / N)
            wconst = cpool.tile([P, P], bf16)
            nc.vector.memset(wconst[:], 0.5)
            wrhs = cpool.tile([P, P], bf16)
            nc.vector.memset(wrhs[:], 0.5)
            wups = wupsum.tile([P, RS], f32, tag="wu", name="wu")
            for _ in range(WARM_PRE):
                nc.tensor.matmul(wups[:, 0:P], wconst[:], wrhs[:])

            # ---------- Dykstra soft-top-k on alpha (serial, tiny) ----------
            # y_t = relu(y0 + c_t), c_{t+1} = c_t + (K - sum(y_t))/N with
            # y_0 = y0 = alpha/l unclipped (aext is pre-scaled by 1/l).
            # c accumulates in PSUM via the PE; t*K/N folds into y0t tiles.
            y0t1 = cpool.tile([P, W32], f32)
            nc.vector.tensor_scalar(
                y0t1[:], al_sb[:], 1.0, KTOP / N, AluOpType.mult, AluOpType.add
            )
            y0 = cpool.tile([P, W32], f32)
            part0 = wpool.tile([P, 1], f32, tag="part", name="part")
            nc.vector.tensor_scalar(
                y0[:], al_sb[:], 1.0, 0.0,
                AluOpType.mult, AluOpType.add, accum_out=part0[:],
            )
            ps = dpsum.tile([P, 1], f32, tag="dps", name="dps")
            nc.tensor.matmul(ps[:], m3[:], part0[:], start=True, stop=False)
            for t in range(1, NITER_DEV):
                c_sb = wpool.tile([P, 1], f32, tag="csb", name="csb")
                nc.vector.tensor_copy(c_sb[:], ps[:])
                cur = wpool.tile([P, W32], f32, tag="cur", name="cur")
                # accum_out reduces with op1, so the relu (max) and the row
                # sum (add) must be two instructions
                nc.vector.tensor_scalar(
                    cur[:], y0t1[:], c_sb[:], 0.0,
                    AluOpType.add, AluOpType.max,
                )
                cur2 = wpool.tile([P, W32], f32, tag="cur2", name="cur2")
                part = wpool.tile([P, 1], f32, tag="part", name="part")
                nc.vector.tensor_scalar(
                    cur2[:], cur[:], 1.0, 0.0,
                    AluOpType.mult, AluOpType.add, accum_out=part[:],
                )
                nc.tensor.matmul(
                    ps[:], m3[:], part[:], start=False, stop=(t == NITER_DEV - 1)
                )
            # c_eff = c + NITER*K/N, the scalar the mask chunks add to y0
            c_eff = cpool.tile([P, 1], f32)
            nc.vector.tensor_scalar(
                c_eff[:], ps[:], 1.0, NITER_DEV * KTOP / N,
                AluOpType.mult, AluOpType.add,
            )

            # ---------- mask + vs production, quad pipeline ----------------
            # All on the vector engine (gpsimd tensor ops are ~15x slower
            # and poison vector via SBUF contention).  Quad granularity
            # amortizes DVE overhead: the four overlapping 512-wide mask
            # windows of one quad are a single op via a 3D access pattern
            # (element (p, u, j) = big[p, 512g + 128u + j]).
            vsqs = []
            for g in range(8):
                bm = bmp.tile([P, 4 * RS], bf16, tag="bm", name=f"bm{g}")
                vsq = vsp.tile([P, 4 * RS], bf16, tag=f"vsq{g}", name=f"vsq{g}")
                halves = 2 if g == 0 else 1
                step = 4 // halves
                for h in range(halves):
                    u0 = h * step
                    bsrc = bass.AP(
                        big[:].tensor, RS * g + P * u0,
                        [[N + RS, P], [P, step], [1, RS]],
                    )
                    bm_h = bm[:, RS * u0 : RS * (u0 + step)]
                    nc.scalar.activation(
                        bm_h.rearrange("p (u j) -> p u j", u=step), bsrc,
                        AFT.Relu, bias=c_eff[:],
                    )
                    nc.vector.tensor_tensor(
                        vsq[:, RS * u0 : RS * (u0 + step)], bm_h,
                        vts[g][:, RS * u0 : RS * (u0 + step)], AluOpType.mult,
                    )
                vsqs.append(vsq)
            vss = [
                vsqs[cb // 4][:, RS * (cb % 4) : RS * (cb % 4 + 1)]
                for cb in range(NCB)
            ]

            # ---------- main matmul stream ----------
            # Trickle phase: banks 0+1 interleaved, two matmuls per vs chunk
            # => PE consumption rate == vs production rate, no idle.
            accs = [
                mpsum.tile([P, RS], f32, tag="acc", name=f"acc{b}")
                for b in range(2)
            ]
            for cb in range(NCB):
                for b in range(2):
                    nc.tensor.matmul(
                        accs[b][:],
                        xts[b][:, P * cb : P * (cb + 1)],
                        vss[cb],
                        start=(cb == 0),
                        stop=(cb == NCB - 1),
                    )
            for b in range(2):
                ot = otp.tile([P, RS], f32, tag="ot", name="ot")
                nc.scalar.activation(ot[:], accs[b][:], AFT.Copy)
                nc.scalar.dma_start(out_d[P * b : P * (b + 1), :], ot[:])
            # Steady phase: banks 2..7, full rate, progressive drain.
            # The last bank accumulates into two half-width PSUM tiles so its
            # drain (the exposed tail) is pipelined in halves.
            for b in range(2, NBB):
                if b < NBB - 1:
                    acc = mpsum.tile([P, RS], f32, tag="acc", name=f"acc{b}")
                    for cb in range(NCB):
                        nc.tensor.matmul(
                            acc[:],
                            xts[b][:, P * cb : P * (cb + 1)],
                            vss[cb],
                            start=(cb == 0),
                            stop=(cb == NCB - 1),
                        )
                    ot = otp.tile([P, RS], f32, tag="ot", name="ot")
                    nc.scalar.activation(ot[:], acc[:], AFT.Copy)
                    nc.scalar.dma_start(out_d[P * b : P * (b + 1), :], ot[:])
                else:
                    ha = fpsum.tile([P, RS // 2], f32, tag="fa", name="fa")
                    hb = fpsum.tile([P, RS // 2], f32, tag="fb", name="fb")
                    for cb in range(NCB):
                        nc.tensor.matmul(
                            hb[:],
                            xts[b][:, P * cb : P * (cb + 1)],
                            vss[cb][:, RS // 2 : RS],
                            start=(cb == 0),
                            stop=(cb == NCB - 1),
                        )
                        nc.tensor.matmul(
                            ha[:],
                            xts[b][:, P * cb : P * (cb + 1)],
                            vss[cb][:, 0 : RS // 2],
                            start=(cb == 0),
                            stop=(cb == NCB - 1),
                        )
                    for half, ht in ((0, ha), (1, hb)):
                        ot = otp.tile([P, RS // 2], f32, tag="oth", name="oth")
                        nc.scalar.activation(ot[:], ht[:], AFT.Copy)
                        nc.scalar.dma_start(
                            out_d[
                                P * b : P * (b + 1),
                                RS // 2 * half : RS // 2 * (half + 1),
                            ],
                            ot[:],
                        )

    nc.compile()
    return nc


def _get_nc():
    if "nc" not in _CACHE:
        _CACHE["nc"] = _build_nc()
    return _CACHE["nc"]


def _prep_inputs(x, V, alpha):
    import ml_dtypes

    bf16 = ml_dtypes.bfloat16
    x = np.asarray(x, dtype=np.float32)
    V = np.asarray(V, dtype=np.float32)
    alpha = np.ascontiguousarray(np.asarray(alpha, dtype=np.float32))
    # batch-major interleave: xb[p, (b*NCB + cb)*P + j] = x[128b+j, 128cb+p]
    xb = np.ascontiguousarray(
        x.astype(bf16)
        .reshape(NBB, P, NCB, P)
        .transpose(3, 0, 2, 1)
        .reshape(P, NBB * NCB * P)
    )
    # r-reversed, doubled V (flip so every device access pattern is positive
    # stride; doubling handles the circulant wrap)
    VTflipbig = np.ascontiguousarray(
        np.concatenate([V.T[:, ::-1], V.T[:, ::-1]], axis=1).astype(bf16)
    )
    in_maps = []
    alpha_rev = alpha[::-1]
    for k in range(NCORES):
        R0 = RS * k
        s = (N - RS - R0) % N
        # vband[c, j] = VTflipbig[c, s + c + j] -- the diagonal band,
        # linearized on host so the device load is a contiguous DMA.
        window = np.lib.stride_tricks.as_strided(
            VTflipbig[:, s:],
            shape=(N, RS),
            strides=(VTflipbig.strides[0] + VTflipbig.strides[1],
                     VTflipbig.strides[1]),
        )
        vb = np.ascontiguousarray(
            window.reshape(NCB, P, RS).transpose(1, 0, 2).reshape(P, NCB * RS)
        )
        # Dykstra is permutation-equivariant: feeding reversed+rolled alpha
        # makes the device compute the r-reversed mask directly.  aext is
        # that alpha pre-scaled by 1/l, linear + wrap for the circulant read.
        al_k = np.roll(alpha_rev, R0 + RS) * INV_L
        aext = np.concatenate([al_k, al_k[: 2 * RS]]).astype(bf16)
        in_maps.append(
            {
                "xb": xb,
                "vb": vb,
                "aext": np.ascontiguousarray(aext),
            }
        )
    return in_maps


def kernel(x, V, alpha, _trace=False, _return_raw=False):
    from concourse.bass_utils import run_bass_kernel_spmd

    nc = _get_nc()
    in_maps = _prep_inputs(x, V, alpha)
    res = run_bass_kernel_spmd(
        nc, in_maps, list(range(NCORES)), trace=_trace
    )
    # per-core outputs come back with the r axis reversed (see _build_nc)
    out = np.concatenate(
        [res.results[k]["out"][:, ::-1] for k in range(NCORES)], axis=1
    )
    if _return_raw:
        return out, res
    return out


if __name__ == "__main__":
    x = np.load(os.path.join(os.path.dirname(__file__), "work/x.npy"))
    V = np.load(os.path.join(os.path.dirname(__file__), "work/V.npy"))
    alpha = np.load(os.path.join(os.path.dirname(__file__), "work/alpha.npy"))
    out = kernel(x, V, alpha)
    exp = np.load(os.path.join(os.path.dirname(__file__), "work/expected.npy"))
    err = np.abs(out - exp)
    print("maxabs", err.max(), "scale-rel", err.max() / np.abs(exp).max())
